# revision 12
# baseline (speedup 1.0000x reference)
"""Trainium2 Bass kernel for the EnhancTrAISformer dense transformer.

Strategy: data-parallel over batch (B=32 -> 4 sequences per core x 8 cores).
All activations are kept in "transposed" layout [C(partitions), tokens] so
every GEMM contracts over the partition dimension with bf16 operands and
fp32 PSUM accumulation. LayerNorm statistics are computed with ones-vector
matmuls on the TensorEngine; softmax runs without max-subtraction (scores
are O(1) for this model) with causal masking via precomputed 0/1 mask tiles
and the softmax denominator comes for free from a ones-column appended to V.
"""
import sys

sys.path.insert(0, '/opt/trn_rl_repo')

import numpy as np
import ml_dtypes

import concourse.bass as bass
import concourse.mybir as mybir
import concourse.tile as tile
from concourse.bass_utils import run_bass_kernel_spmd
from concourse.vector_clock import ScopedClock

BF16 = mybir.dt.bfloat16
F32 = mybir.dt.float32
AF = mybir.ActivationFunctionType
OP = mybir.AluOpType
nbf = ml_dtypes.bfloat16

# model dims
B, T, C, L, H, D = 32, 1024, 768, 8, 8, 96
DFF = 4 * C
FULL = 622
EPS = 1e-5
N_CORES = 8
SEQ_PER_CORE = B // N_CORES          # 4
TOK = SEQ_PER_CORE * T               # 4096 tokens per core
NC_CHUNK = TOK // 512                # 8 chunks of 512 tokens
KT = C // 128                        # 6
MT1 = DFF // 128                     # 24
ATT_SIZES = (250, 270, 30, 72)
EMB_SIZES = (256, 256, 128, 128)

# table groups: (attr, kt within table, C-tile index of output)
TAB_GROUPS = [  # (attr, global group idx) ; attr -> vocab tiles
    (0, [0, 1]),   # lat: C-tiles 0,1 (emb 256)
    (1, [2, 3, 4]),  # lon: C-tiles 2,3
    (2, [5]),      # sog: C-tile 4
    (3, [6]),      # cog: C-tile 5
]


def _patched_drain_and_barrier(self, tick_clock, wait_clock):
    """Walrus on this image only allows 1 sync-wait on a CTRL/Drain inst;
    split the tile-exit drain's waits across multiple drains."""
    nc = self.nc
    drain_inst = nc.sync.drain()
    wait_clock.add_sem_waits(
        drain_inst.ins, ScopedClock({None: tick_clock.global_clock})
    )
    si = drain_inst.ins.sync_info
    MAXW = 1
    if si is not None and len(si.on_wait) > MAXW:
        waits = list(si.on_wait)
        si.on_wait = waits[:MAXW]
        drain_inst.ins.sync_info = si
        rest = waits[MAXW:]
        while rest:
            chunk, rest = rest[:MAXW], rest[MAXW:]
            extra = nc.sync.drain()
            esi = extra.ins.sync_info
            if esi is None:
                esi = mybir.SyncInfo(on_wait=chunk, on_update=[])
            else:
                esi.on_wait = chunk
            extra.ins.sync_info = esi

    nc.all_engine_barrier()
    assert self.sems is not None
    popped = nc._tile_sem_poison_stack.pop()
    assert popped is self._sem_poison
    nc.clear_and_free_semaphores(list(self.sems.allocated().values()))
    nc.all_engine_barrier()


tile.TileContext._drain_and_barrier = _patched_drain_and_barrier

# Walrus on this image caps sync-waits per instruction (1 for CTRL/Drain,
# ~2 for TPB compute). Split excess waits onto NoOps inserted just before
# the overloaded instruction (same engine, in-order => semantics preserved).
import orjson as _orjson

_orig_to_json_bytes = bass.Bass.to_json_bytes
_wsplit_n = [0]


def _split_waits_json(data):
    for fn in data.get('functions', []):
        for blk in fn.get('blocks', []):
            insts = blk.get('instructions', [])
            out = []
            changed = False
            for inst in insts:
                si = inst.get('sync_info')
                waits = (si or {}).get('on_wait') or []
                maxw = 1 if inst.get('opcode') in ('Drain', 'NoOp') else 2
                if len(waits) > maxw:
                    changed = True
                    excess = waits[:-maxw] if maxw else waits
                    si['on_wait'] = waits[-maxw:] if maxw else []
                    for w in excess:
                        _wsplit_n[0] += 1
                        out.append({
                            'name': f'I-wsplit{_wsplit_n[0]}',
                            'opcode': 'NoOp',
                            'engine': inst['engine'],
                            'ins': [], 'outs': [],
                            'debug': inst.get('debug', 0),
                            'sync_info': {'on_update': [], 'on_wait': [w]},
                        })
                out.append(inst)
            if changed:
                blk['instructions'] = out
    return data


def _patched_to_json_bytes(self):
    data = _orjson.loads(_orig_to_json_bytes(self))
    _split_waits_json(data)
    return _orjson.dumps(data)


bass.Bass.to_json_bytes = _patched_to_json_bytes



# ---------------------------------------------------------------- host prep
def prep_params(params):
    """Fold LN affine params into weights/biases; build device layouts."""
    p = {k: np.asarray(v, np.float32) for k, v in params.items()
         if not isinstance(v, dict)}
    blk = {k: np.asarray(v, np.float32) for k, v in params['blocks'].items()}

    sc = 1.0 / np.sqrt(D)
    out = {}

    wqkv = np.zeros((L, 3, 128, KT, C), nbf)
    wo = np.zeros((L, 96, H, C), nbf)
    w1 = np.zeros((L, MT1, 128, KT, 128), nbf)
    w2 = np.zeros((L, KT, 128, MT1, 128), nbf)
    biasA = np.zeros((L, 128, 52), np.float32)
    vbias_rows = np.zeros((L, 1, C), nbf)
    has_vbias = False

    # permutation for V output columns: j = d*8 + h  <-  h*96 + d
    vperm = np.empty(C, np.int64)
    for j in range(C):
        d_, h_ = j // 8, j % 8
        vperm[d_ * 8 + h_] = h_ * 96 + d_
    # careful: vperm[j] gives source col for dest col j
    vperm = np.array([( (j % 8) * 96 + (j // 8)) for j in range(C)])

    for l in range(L):
        g1, b1v = blk['ln1_g'][l], blk['ln1_b'][l]
        g2, b2v = blk['ln2_g'][l], blk['ln2_b'][l]
        Wq, Wk, Wv, Wo = blk['Wq'][l], blk['Wk'][l], blk['Wv'][l], blk['Wo'][l]
        W1, W2 = blk['W1'][l], blk['W2'][l]
        bq = blk['bq'][l] + b1v @ Wq
        bk = (blk['bk'][l] + b1v @ Wk) * sc
        bv = blk['bv'][l] + b1v @ Wv
        bo = blk['bo'][l]
        bm1 = blk['b1'][l] + b2v @ W1
        bm2 = blk['b2'][l]
        Wq_e = g1[:, None] * Wq
        Wk_e = (g1[:, None] * Wk) * sc
        Wv_e = (g1[:, None] * Wv)[:, vperm]
        bv_e = bv[vperm]
        W1_e = g2[:, None] * W1

        wqkv[l, 0] = Wq_e.reshape(KT, 128, C).transpose(1, 0, 2).astype(nbf)
        wqkv[l, 1] = Wk_e.reshape(KT, 128, C).transpose(1, 0, 2).astype(nbf)
        wqkv[l, 2] = Wv_e.reshape(KT, 128, C).transpose(1, 0, 2).astype(nbf)
        wo[l] = Wo.reshape(H, 96, C).transpose(1, 0, 2).astype(nbf)
        # w1[l, m, p, kt, f] = W1_e[kt*128+p, m*128+f]
        w1[l] = W1_e.reshape(KT, 128, MT1, 128).transpose(2, 1, 0, 3).astype(nbf)
        # w2[l, mo, p, kt, f] = W2[kt*128+p, mo*128+f]  (kt here is over DFF: 24)
        w2[l] = W2.reshape(MT1, 128, KT, 128).transpose(2, 1, 0, 3).astype(nbf)
        # biases: qb_hd [128,8] (d-major), kb_hd, ob [128,6], b1 [128,24], b2 [128,6]
        qb = np.zeros((128, H), np.float32)
        kb = np.zeros((128, H), np.float32)
        qb[:96] = bq.reshape(H, 96).T
        kb[:96] = bk.reshape(H, 96).T
        biasA[l, :, 0:8] = qb
        biasA[l, :, 8:16] = kb
        biasA[l, :, 16:22] = bo.reshape(KT, 128).T
        biasA[l, :, 22:46] = bm1.reshape(MT1, 128).T
        biasA[l, :, 46:52] = bm2.reshape(KT, 128).T
        if np.any(bv_e != 0):
            has_vbias = True
        vbias_rows[l, 0] = bv_e.astype(nbf)

    gf, bf = p['lnf_g'], p['lnf_b']
    Wh = p['Whead']
    Wh_e = gf[:, None] * Wh
    headb = bf @ Wh
    whead = Wh_e.reshape(KT, 128, FULL).transpose(1, 0, 2).astype(nbf)
    has_headb = bool(np.any(headb != 0))
    headb_row = headb[None, :].astype(nbf)

    # tables (padded to 128-row tiles), lat/lon emb 256 cols, sog/cog 128
    tabs = np.zeros((128, 7, 256), nbf)
    iota = np.zeros((128, 8), np.float32)
    tbls = [p['lat_emb'], p['lon_emb'], p['sog_emb'], p['cog_emb']]
    g = 0
    for a, tb in enumerate(tbls):
        nv, ne = tb.shape
        nkt = (nv + 127) // 128
        for k in range(nkt):
            rows = tb[k * 128:(k + 1) * 128]
            tabs[:rows.shape[0], g, :ne] = rows.astype(nbf)
            iota[:, g] = k * 128 + np.arange(128)
            g += 1
    assert g == 7

    pos = p['pos_emb'][0]  # [T, C]
    pos_t = pos.reshape(T, KT, 128).transpose(2, 1, 0).astype(np.float32)
    pos_t = np.ascontiguousarray(pos_t)  # [128, 6, 1024]

    kl = np.arange(128)[:, None]
    ql = np.arange(128)[None, :]
    masks = (ql >= kl).astype(nbf)  # [128,128] lower-causal in [k,q] layout

    out.update(wqkv=wqkv, wo=wo, w1=w1, w2=w2, whead=whead, biasA=biasA,
               tabs=tabs, iota=iota, pos=pos_t, masks=masks,
               vbias=vbias_rows, headb=headb_row,
               has_vbias=has_vbias, has_headb=has_headb)
    return out


# ---------------------------------------------------------------- builder
def build(n_layers=L, has_vbias=False, has_headb=False):
    nc = bass.Bass()
    d = {}
    d['wqkv'] = nc.declare_dram_parameter('wqkv', [L, 3, 128, KT, C], BF16)
    d['wo'] = nc.declare_dram_parameter('wo', [L, 96, H, C], BF16)
    d['w1'] = nc.declare_dram_parameter('w1', [L, MT1, 128, KT, 128], BF16)
    d['w2'] = nc.declare_dram_parameter('w2', [L, KT, 128, MT1, 128], BF16)
    d['whead'] = nc.declare_dram_parameter('whead', [128, KT, FULL], BF16)
    d['biasA'] = nc.declare_dram_parameter('biasA', [L, 128, 52], F32)
    d['tabs'] = nc.declare_dram_parameter('tabs', [128, 7, 256], BF16)
    d['iota'] = nc.declare_dram_parameter('iota', [128, 8], F32)
    d['pos'] = nc.declare_dram_parameter('pos', [128, KT, T], F32)
    d['masks'] = nc.declare_dram_parameter('masks', [128, 128], BF16)
    d['idx'] = nc.declare_dram_parameter('idx', [32, 512], F32)
    if has_vbias:
        d['vbias'] = nc.declare_dram_parameter('vbias', [L, 1, C], BF16)
    if has_headb:
        d['headb'] = nc.declare_dram_parameter('headb', [1, FULL], BF16)
    out_d = nc.declare_dram_parameter('out', [32, 128, FULL], F32, isOutput=True)
    hbuf = nc.dram_tensor('hbuf', [KT, 128, TOK], F32)

    with tile.TileContext(nc) as tc:
        from contextlib import ExitStack
        with ExitStack() as ctx:
            sb = ctx.enter_context(tc.tile_pool(name='sb', bufs=1))
            sb2 = ctx.enter_context(tc.tile_pool(name='sb2', bufs=2))
            sb3 = ctx.enter_context(tc.tile_pool(name='sb3', bufs=3))
            sb4 = ctx.enter_context(tc.tile_pool(name='sb4', bufs=3))
            stats = ctx.enter_context(tc.tile_pool(name='stats', bufs=4))
            ln7 = ctx.enter_context(tc.tile_pool(name='ln7', bufs=6))
            gel = ctx.enter_context(tc.tile_pool(name='gel', bufs=24))
            pp_ = ctx.enter_context(tc.tile_pool(name='pp', bufs=4))
            ps_small = ctx.enter_context(
                tc.tile_pool(name='ps_s', bufs=2, space='PSUM'))
            ps_b = ctx.enter_context(
                tc.tile_pool(name='ps_b', bufs=2, space='PSUM'))
            ps_mm = ctx.enter_context(
                tc.tile_pool(name='ps_mm', bufs=4, space='PSUM'))

            # constants
            onesC = sb.tile([128, 1], F32, tag='onesC')
            nc.vector.memset(onesC[:], 1.0 / C)
            ones_row = sb.tile([1, 128], F32, tag='ones_row')
            nc.vector.memset(ones_row[:], 1.0)
            ones_row_bf = sb.tile([1, 128], BF16, tag='ones_row_bf')
            nc.vector.memset(ones_row_bf[:], 1.0)
            eps_sb = sb.tile([1, 1], F32, tag='eps')
            nc.vector.memset(eps_sb[:], EPS)

            masks_sb = sb.tile([128, 128], BF16, tag='masks')
            nc.sync.dma_start(out=masks_sb[:], in_=d['masks'][:])
            tabs_sb = sb.tile([128, 7, 256], BF16, tag='tabs')
            nc.sync.dma_start(out=tabs_sb[:], in_=d['tabs'][:])
            iota_sb = sb.tile([128, 8], F32, tag='iota')
            nc.sync.dma_start(out=iota_sb[:], in_=d['iota'][:])

            # ---------------- embedding ----------------
            for c in range(NC_CHUNK):
                csl = slice(c * 512, (c + 1) * 512)
                ct_base = 0
                for a, groups in TAB_GROUPS:
                    r = a * 8 + c
                    idxb = sb2.tile([128, 512], F32, tag='idxb')
                    nc.sync.dma_start(
                        out=idxb[:], in_=d['idx'][r:r + 1, :].broadcast_to((128, 512)))
                    ohs = []
                    for g in groups:
                        oh = sb2.tile([128, 512], BF16, tag='onehot')
                        nc.vector.tensor_scalar(
                            out=oh[:], in0=idxb[:],
                            scalar1=iota_sb[:, g:g + 1], scalar2=None,
                            op0=OP.is_equal)
                        ohs.append(oh)
                    n_m = EMB_SIZES[a] // 128
                    for mi in range(n_m):
                        pe = ps_mm.tile([128, 512], F32, tag='ps_mm')
                        for j, (g, oh) in enumerate(zip(groups, ohs)):
                            nc.tensor.matmul(
                                pe[:], tabs_sb[:, g, mi * 128:(mi + 1) * 128],
                                oh[:], start=(j == 0), stop=(j == len(groups) - 1))
                        ct = ct_base + mi
                        post = sb4.tile([128, 512], F32, tag='hin')
                        nc.sync.dma_start(
                            out=post[:],
                            in_=d['pos'][:, ct, (c % 2) * 512:(c % 2) * 512 + 512])
                        h0 = sb4.tile([128, 512], F32, tag='hin')
                        nc.vector.scalar_tensor_tensor(
                            out=h0[:], in0=pe[:], scalar=1.0, in1=post[:],
                            op0=OP.mult, op1=OP.add)
                        nc.sync.dma_start(out=hbuf[ct_base + mi, :, csl], in_=h0[:])
                    ct_base += n_m

            whead_sb = sb.tile([128, KT, FULL], BF16, tag='tabs')
            for kt in range(KT):
                nc.sync.dma_start(out=whead_sb[:, kt, :], in_=d['whead'][:, kt, :])
            if has_headb:
                headb_sb = sb.tile([1, FULL], BF16, tag='headb')
                nc.sync.dma_start(out=headb_sb[:], in_=d['headb'][:])

            # ---------------- helper: layernorm into xn tile ----------------
            def layer_norm(c, xn):
                """Reads hbuf chunk c, writes standardized bf16 into xn
                ([128, KT, 512]). Returns nothing."""
                csl = slice(c * 512, (c + 1) * 512)
                hts = []
                mu_ps = ps_small.tile([1, 512], F32, tag='ps_small')
                ex2_ps = ps_small.tile([1, 512], F32, tag='ps_small')
                for kt in range(KT):
                    ht = ln7.tile([128, 512], F32, tag='hln')
                    nc.sync.dma_start(out=ht[:], in_=hbuf[kt, :, csl])
                    hts.append(ht)
                    nc.tensor.matmul(mu_ps[:], onesC[:], ht[:],
                                     start=(kt == 0), stop=(kt == KT - 1))
                for kt in range(KT):
                    x2 = sb2.tile([128, 512], F32, tag='x2')
                    nc.vector.scalar_tensor_tensor(
                        out=x2[:], in0=hts[kt][:], scalar=1.0, in1=hts[kt][:],
                        op0=OP.mult, op1=OP.mult)
                    nc.tensor.matmul(ex2_ps[:], onesC[:], x2[:],
                                     start=(kt == 0), stop=(kt == KT - 1))
                mu_sb = stats.tile([1, 512], F32, tag='st')
                nc.vector.tensor_copy(mu_sb[:], mu_ps[:])
                t1 = stats.tile([1, 512], F32, tag='st')
                nc.vector.tensor_tensor(out=t1[:], in0=mu_sb[:], in1=mu_sb[:],
                                        op=OP.mult)
                var = stats.tile([1, 512], F32, tag='st')
                nc.vector.tensor_tensor(out=var[:], in0=ex2_ps[:], in1=t1[:],
                                        op=OP.subtract)
                std = stats.tile([1, 512], F32, tag='st')
                nc.scalar.activation(std[:], var[:], AF.Sqrt, bias=eps_sb[:])
                inv = stats.tile([1, 512], F32, tag='st')
                nc.vector.reciprocal(inv[:], std[:])
                mub = ps_b.tile([128, 512], F32, tag='ps_bc')
                nc.tensor.matmul(mub[:], ones_row[:], mu_sb[:])
                sbc = ps_b.tile([128, 512], F32, tag='ps_bc')
                nc.tensor.matmul(sbc[:], ones_row[:], inv[:])
                for kt in range(KT):
                    cent = sb2.tile([128, 512], F32, tag='x2')
                    nc.vector.scalar_tensor_tensor(
                        out=cent[:], in0=hts[kt][:], scalar=1.0, in1=mub[:],
                        op0=OP.mult, op1=OP.subtract)
                    nc.vector.tensor_tensor(out=xn[:, kt, :], in0=cent[:],
                                            in1=sbc[:], op=OP.mult)

            # ---------------- layers ----------------
            for l in range(n_layers):
                wq_sb = sb.tile([128, KT, C], BF16, tag='wq')
                wk_sb = sb.tile([128, KT, C], BF16, tag='wk')
                wv_sb = sb.tile([128, KT, C], BF16, tag='wv')
                for kt in range(KT):
                    nc.sync.dma_start(out=wq_sb[:, kt, :], in_=d['wqkv'][l, 0, :, kt, :])
                    nc.sync.dma_start(out=wk_sb[:, kt, :], in_=d['wqkv'][l, 1, :, kt, :])
                    nc.sync.dma_start(out=wv_sb[:, kt, :], in_=d['wqkv'][l, 2, :, kt, :])
                bias_sb = sb2.tile([128, 52], F32, tag='biasA')
                nc.sync.dma_start(out=bias_sb[:], in_=d['biasA'][l])
                if has_vbias:
                    vb = sb2.tile([1, C], BF16, tag='vbias')
                    nc.sync.dma_start(out=vb[:], in_=d['vbias'][l])

                # ---- attention, per sequence ----
                for s in range(SEQ_PER_CORE):
                    q_hd = sb.tile([128, H, T], BF16, tag='q_hd')
                    k_hd = sb.tile([128, H, T], BF16, tag='k_hd')
                    v_sb = sb.tile([128, 8, 776], BF16, tag='v_sb')
                    y_hd = sb.tile([128, H, T], BF16, tag='y_hd')
                    nc.vector.memset(v_sb[:, :, 768:776], 1.0)
                    for cc in range(2):
                        c = s * 2 + cc
                        xn = sb3.tile([128, KT, 512], BF16, tag='xn')
                        layer_norm(c, xn)
                        tsl = slice(cc * 512, cc * 512 + 512)
                        for h in range(H):
                            qp = ps_mm.tile([128, 512], F32, tag='ps_mm')
                            for kt in range(KT):
                                nc.tensor.matmul(
                                    qp[0:96, :],
                                    wq_sb[:, kt, h * 96:(h + 1) * 96],
                                    xn[:, kt, :],
                                    start=(kt == 0), stop=(kt == KT - 1))
                            nc.vector.tensor_scalar(
                                out=q_hd[0:96, h, tsl], in0=qp[0:96, :],
                                scalar1=bias_sb[0:96, h:h + 1], scalar2=None,
                                op0=OP.add)
                            kp = ps_mm.tile([128, 512], F32, tag='ps_mm')
                            for kt in range(KT):
                                nc.tensor.matmul(
                                    kp[0:96, :],
                                    wk_sb[:, kt, h * 96:(h + 1) * 96],
                                    xn[:, kt, :],
                                    start=(kt == 0), stop=(kt == KT - 1))
                            nc.vector.tensor_scalar(
                                out=k_hd[0:96, h, tsl], in0=kp[0:96, :],
                                scalar1=bias_sb[0:96, 8 + h:9 + h], scalar2=None,
                                op0=OP.add)
                        for tm in range(4):
                            for (n0, nw) in ((0, 512), (512, 256)):
                                vp = ps_mm.tile([128, 512], F32, tag='ps_mm')
                                for kt in range(KT):
                                    nc.tensor.matmul(
                                        vp[:, 0:nw],
                                        xn[:, kt, tm * 128:(tm + 1) * 128],
                                        wv_sb[:, kt, n0:n0 + nw],
                                        start=(kt == 0),
                                        stop=(kt == KT - 1 if not has_vbias else False))
                                if has_vbias:
                                    nc.tensor.matmul(
                                        vp[:, 0:nw], ones_row_bf[:],
                                        vb[:, n0:n0 + nw],
                                        start=False, stop=True)
                                nc.vector.tensor_copy(
                                    v_sb[:, cc * 4 + tm, n0:n0 + nw], vp[:, 0:nw])
                    # attention proper
                    for h in range(H):
                        for qn in range(2):
                            nkt = 4 * (qn + 1)
                            qsl = slice(qn * 512, qn * 512 + 512)
                            yp = ps_mm.tile([128, 512], F32, tag='ps_mm')
                            for kt in range(nkt):
                                sp = ps_mm.tile([128, 512], F32, tag='ps_mm')
                                nc.tensor.matmul(
                                    sp[:], k_hd[0:96, h, kt * 128:(kt + 1) * 128],
                                    q_hd[0:96, h, qsl])
                                p_t = pp_.tile([128, 512], BF16, tag='p')
                                nc.scalar.activation(p_t[:], sp[:], AF.Exp)
                                j = kt - 4 * qn
                                if 0 <= j:
                                    nc.vector.tensor_tensor(
                                        out=p_t[:, 128 * j:128 * j + 128],
                                        in0=p_t[:, 128 * j:128 * j + 128],
                                        in1=masks_sb[:], op=OP.mult)
                                    if j > 0:
                                        nc.vector.memset(p_t[:, 0:128 * j], 0.0)
                                nc.tensor.matmul(
                                    yp[0:97, :], v_sb[:, kt, h:776:8], p_t[:],
                                    start=(kt == 0), stop=(kt == nkt - 1))
                            rec = stats.tile([1, 512], F32, tag='st')
                            nc.vector.reciprocal(rec[:], yp[96:97, :])
                            rb = ps_b.tile([128, 512], F32, tag='ps_bc')
                            nc.tensor.matmul(rb[0:96, :], ones_row[0:1, 0:96], rec[:])
                            rbs = sb2.tile([128, 512], F32, tag='rbs')
                            nc.vector.tensor_copy(rbs[0:96, :], rb[0:96, :])
                            nc.vector.tensor_tensor(
                                out=y_hd[0:96, h, qsl], in0=yp[0:96, :],
                                in1=rbs[0:96, :], op=OP.mult)
                    # projection + residual
                    for qn in range(2):
                        c = s * 2 + qn
                        csl = slice(c * 512, (c + 1) * 512)
                        qsl = slice(qn * 512, qn * 512 + 512)
                        for mo in range(KT):
                            wos = sb3.tile([96, H, 128], BF16, tag='wos')
                            nc.sync.dma_start(
                                out=wos[:], in_=d['wo'][l, :, :, mo * 128:(mo + 1) * 128])
                            pj = ps_mm.tile([128, 512], F32, tag='ps_mm')
                            for ht in range(H):
                                nc.tensor.matmul(
                                    pj[:], wos[:, ht, :],
                                    y_hd[0:96, ht, qsl],
                                    start=(ht == 0), stop=(ht == H - 1))
                            hres = sb4.tile([128, 512], F32, tag='hin')
                            nc.sync.dma_start(out=hres[:], in_=hbuf[mo, :, csl])
                            hnew = sb4.tile([128, 512], F32, tag='hin')
                            nc.vector.scalar_tensor_tensor(
                                out=hnew[:], in0=pj[:],
                                scalar=bias_sb[:, 16 + mo:17 + mo], in1=hres[:],
                                op0=OP.add, op1=OP.add)
                            nc.sync.dma_start(out=hbuf[mo, :, csl], in_=hnew[:])

                # ---- MLP, per 512-token chunk ----
                for c in range(NC_CHUNK):
                    csl = slice(c * 512, (c + 1) * 512)
                    xn = sb3.tile([128, KT, 512], BF16, tag='xn')
                    layer_norm(c, xn)
                    gts = []
                    for m in range(MT1):
                        w1s = sb4.tile([128, KT, 128], BF16, tag='w1s')
                        nc.sync.dma_start(out=w1s[:], in_=d['w1'][l, m])
                        mp = ps_mm.tile([128, 512], F32, tag='ps_mm')
                        for kt in range(KT):
                            nc.tensor.matmul(mp[:], w1s[:, kt, :], xn[:, kt, :],
                                             start=(kt == 0), stop=(kt == KT - 1))
                        g = gel.tile([128, 512], BF16, tag='g')
                        nc.scalar.activation(g[:], mp[:], AF.Gelu,
                                             bias=bias_sb[:, 22 + m:23 + m])
                        gts.append(g)
                    for mo in range(KT):
                        w2s = sb2.tile([128, MT1, 128], BF16, tag='w2s')
                        nc.sync.dma_start(out=w2s[:], in_=d['w2'][l, mo])
                        op_ = ps_mm.tile([128, 512], F32, tag='ps_mm')
                        for kt in range(MT1):
                            nc.tensor.matmul(op_[:], w2s[:, kt, :], gts[kt][:],
                                             start=(kt == 0), stop=(kt == MT1 - 1))
                        hres = sb4.tile([128, 512], F32, tag='hin')
                        nc.sync.dma_start(out=hres[:], in_=hbuf[mo, :, csl])
                        hnew = sb4.tile([128, 512], F32, tag='hin')
                        nc.vector.scalar_tensor_tensor(
                            out=hnew[:], in0=op_[:],
                            scalar=bias_sb[:, 46 + mo:47 + mo], in1=hres[:],
                            op0=OP.add, op1=OP.add)
                        nc.sync.dma_start(out=hbuf[mo, :, csl], in_=hnew[:])

            # ---------------- final LN + head ----------------
            for c in range(NC_CHUNK):
                xn = sb3.tile([128, KT, 512], BF16, tag='xn')
                layer_norm(c, xn)
                for tm in range(4):
                    ob_t = sb.tile([128, FULL], F32, tag='outsb')
                    for (n0, nw) in ((0, 512), (512, FULL - 512)):
                        hp = ps_mm.tile([128, 512], F32, tag='ps_mm')
                        for kt in range(KT):
                            nc.tensor.matmul(
                                hp[:, 0:nw],
                                xn[:, kt, tm * 128:(tm + 1) * 128],
                                whead_sb[:, kt, n0:n0 + nw],
                                start=(kt == 0),
                                stop=(kt == KT - 1 if not has_headb else False))
                        if has_headb:
                            nc.tensor.matmul(hp[:, 0:nw], ones_row_bf[:],
                                             headb_sb[:, n0:n0 + nw],
                                             start=False, stop=True)
                        nc.vector.tensor_copy(ob_t[:, n0:n0 + nw], hp[:, 0:nw])
                    nc.sync.dma_start(out=out_d[c * 4 + tm], in_=ob_t[:])

    return nc


_CACHE = {}
_LAST_RESULT = None


def kernel(x, params):
    x = np.asarray(x, np.float32)
    prep = prep_params(params)
    key = (L, prep['has_vbias'], prep['has_headb'])
    if key not in _CACHE:
        _CACHE[key] = build(L, prep['has_vbias'], prep['has_headb'])
    nc = _CACHE[key]

    idx_full = np.floor(x * np.asarray(ATT_SIZES, np.float32)).astype(np.float32)
    # reference casts via .astype(int32) which truncates; x>=0 so floor==trunc
    in_maps = []
    for core in range(N_CORES):
        xs = idx_full[core * SEQ_PER_CORE:(core + 1) * SEQ_PER_CORE]  # [4,1024,4]
        flat = xs.reshape(TOK, 4).T  # [4, 4096] attr-major
        idx_rows = flat.reshape(4, NC_CHUNK, 512).reshape(32, 512)
        m = dict(wqkv=prep['wqkv'], wo=prep['wo'], w1=prep['w1'], w2=prep['w2'],
                 whead=prep['whead'], biasA=prep['biasA'], tabs=prep['tabs'],
                 iota=prep['iota'], pos=prep['pos'], masks=prep['masks'],
                 idx=np.ascontiguousarray(idx_rows))
        if prep['has_vbias']:
            m['vbias'] = prep['vbias']
        if prep['has_headb']:
            m['headb'] = prep['headb']
        in_maps.append(m)

    import os
    trace = bool(os.environ.get("BASS_TRACE"))
    res = run_bass_kernel_spmd(nc, in_maps, list(range(N_CORES)), trace=trace)
    global _LAST_RESULT
    _LAST_RESULT = res
    outs = []
    for core in range(N_CORES):
        o = res.results[core]['out']  # [32, 128, 622]
        outs.append(o.reshape(SEQ_PER_CORE, T, FULL))
    return np.concatenate(outs, axis=0)


# revision 15
# speedup vs baseline: 1.0028x; 1.0028x over previous
"""Trainium2 Bass kernel for the EnhancTrAISformer dense transformer.

Strategy: data-parallel over batch (B=32 -> 4 sequences per core x 8 cores).
All activations are kept in "transposed" layout [C(partitions), tokens] so
every GEMM contracts over the partition dimension with bf16 operands and
fp32 PSUM accumulation. LayerNorm statistics are computed with ones-vector
matmuls on the TensorEngine; softmax runs without max-subtraction (scores
are O(1) for this model) with causal masking via precomputed 0/1 mask tiles
and the softmax denominator comes for free from a ones-column appended to V.
"""
import sys

sys.path.insert(0, '/opt/trn_rl_repo')

import numpy as np
import ml_dtypes

import concourse.bass as bass
import concourse.mybir as mybir
import concourse.tile as tile
from concourse.bass_utils import run_bass_kernel_spmd
from concourse.vector_clock import ScopedClock

BF16 = mybir.dt.bfloat16
F32 = mybir.dt.float32
AF = mybir.ActivationFunctionType
OP = mybir.AluOpType
nbf = ml_dtypes.bfloat16

# model dims
B, T, C, L, H, D = 32, 1024, 768, 8, 8, 96
DFF = 4 * C
FULL = 622
EPS = 1e-5
N_CORES = 8
SEQ_PER_CORE = B // N_CORES          # 4
TOK = SEQ_PER_CORE * T               # 4096 tokens per core
NC_CHUNK = TOK // 512                # 8 chunks of 512 tokens
KT = C // 128                        # 6
MT1 = DFF // 128                     # 24
ATT_SIZES = (250, 270, 30, 72)
EMB_SIZES = (256, 256, 128, 128)

# table groups: (attr, kt within table, C-tile index of output)
TAB_GROUPS = [  # (attr, global group idx) ; attr -> vocab tiles
    (0, [0, 1]),   # lat: C-tiles 0,1 (emb 256)
    (1, [2, 3, 4]),  # lon: C-tiles 2,3
    (2, [5]),      # sog: C-tile 4
    (3, [6]),      # cog: C-tile 5
]


def _patched_drain_and_barrier(self, tick_clock, wait_clock):
    """Walrus on this image only allows 1 sync-wait on a CTRL/Drain inst;
    split the tile-exit drain's waits across multiple drains."""
    nc = self.nc
    drain_inst = nc.sync.drain()
    wait_clock.add_sem_waits(
        drain_inst.ins, ScopedClock({None: tick_clock.global_clock})
    )
    si = drain_inst.ins.sync_info
    MAXW = 1
    if si is not None and len(si.on_wait) > MAXW:
        waits = list(si.on_wait)
        si.on_wait = waits[:MAXW]
        drain_inst.ins.sync_info = si
        rest = waits[MAXW:]
        while rest:
            chunk, rest = rest[:MAXW], rest[MAXW:]
            extra = nc.sync.drain()
            esi = extra.ins.sync_info
            if esi is None:
                esi = mybir.SyncInfo(on_wait=chunk, on_update=[])
            else:
                esi.on_wait = chunk
            extra.ins.sync_info = esi

    nc.all_engine_barrier()
    assert self.sems is not None
    popped = nc._tile_sem_poison_stack.pop()
    assert popped is self._sem_poison
    nc.clear_and_free_semaphores(list(self.sems.allocated().values()))
    nc.all_engine_barrier()


tile.TileContext._drain_and_barrier = _patched_drain_and_barrier

# Walrus on this image caps sync-waits per instruction (1 for CTRL/Drain,
# ~2 for TPB compute). Split excess waits onto NoOps inserted just before
# the overloaded instruction (same engine, in-order => semantics preserved).
import orjson as _orjson

_orig_to_json_bytes = bass.Bass.to_json_bytes
_wsplit_n = [0]


def _split_waits_json(data):
    for fn in data.get('functions', []):
        for blk in fn.get('blocks', []):
            insts = blk.get('instructions', [])
            out = []
            changed = False
            for inst in insts:
                si = inst.get('sync_info')
                waits = (si or {}).get('on_wait') or []
                maxw = 1 if inst.get('opcode') in ('Drain', 'NoOp') else 2
                if len(waits) > maxw:
                    changed = True
                    excess = waits[:-maxw] if maxw else waits
                    si['on_wait'] = waits[-maxw:] if maxw else []
                    for w in excess:
                        _wsplit_n[0] += 1
                        out.append({
                            'name': f'I-wsplit{_wsplit_n[0]}',
                            'opcode': 'NoOp',
                            'engine': inst['engine'],
                            'ins': [], 'outs': [],
                            'debug': inst.get('debug', 0),
                            'sync_info': {'on_update': [], 'on_wait': [w]},
                        })
                out.append(inst)
            if changed:
                blk['instructions'] = out
    return data


def _patched_to_json_bytes(self):
    data = _orjson.loads(_orig_to_json_bytes(self))
    _split_waits_json(data)
    return _orjson.dumps(data)


bass.Bass.to_json_bytes = _patched_to_json_bytes



# ---------------------------------------------------------------- host prep
def prep_params(params):
    """Fold LN affine params into weights/biases; build device layouts."""
    p = {k: np.asarray(v, np.float32) for k, v in params.items()
         if not isinstance(v, dict)}
    blk = {k: np.asarray(v, np.float32) for k, v in params['blocks'].items()}

    sc = 1.0 / np.sqrt(D)
    out = {}

    wqkv = np.zeros((L, 3, 128, KT, C), nbf)
    wo = np.zeros((L, 96, H, C), nbf)
    w1 = np.zeros((L, MT1, 128, KT, 128), nbf)
    w2 = np.zeros((L, KT, 128, MT1, 128), nbf)
    biasA = np.zeros((L, 128, 52), np.float32)
    vbias_rows = np.zeros((L, 1, C), nbf)
    has_vbias = False

    # permutation for V output columns: j = d*8 + h  <-  h*96 + d
    vperm = np.empty(C, np.int64)
    for j in range(C):
        d_, h_ = j // 8, j % 8
        vperm[d_ * 8 + h_] = h_ * 96 + d_
    # careful: vperm[j] gives source col for dest col j
    vperm = np.array([( (j % 8) * 96 + (j // 8)) for j in range(C)])

    for l in range(L):
        g1, b1v = blk['ln1_g'][l], blk['ln1_b'][l]
        g2, b2v = blk['ln2_g'][l], blk['ln2_b'][l]
        Wq, Wk, Wv, Wo = blk['Wq'][l], blk['Wk'][l], blk['Wv'][l], blk['Wo'][l]
        W1, W2 = blk['W1'][l], blk['W2'][l]
        bq = blk['bq'][l] + b1v @ Wq
        bk = (blk['bk'][l] + b1v @ Wk) * sc
        bv = blk['bv'][l] + b1v @ Wv
        bo = blk['bo'][l]
        bm1 = blk['b1'][l] + b2v @ W1
        bm2 = blk['b2'][l]
        Wq_e = g1[:, None] * Wq
        Wk_e = (g1[:, None] * Wk) * sc
        Wv_e = (g1[:, None] * Wv)[:, vperm]
        bv_e = bv[vperm]
        W1_e = g2[:, None] * W1

        wqkv[l, 0] = Wq_e.reshape(KT, 128, C).transpose(1, 0, 2).astype(nbf)
        wqkv[l, 1] = Wk_e.reshape(KT, 128, C).transpose(1, 0, 2).astype(nbf)
        wqkv[l, 2] = Wv_e.reshape(KT, 128, C).transpose(1, 0, 2).astype(nbf)
        wo[l] = Wo.reshape(H, 96, C).transpose(1, 0, 2).astype(nbf)
        # w1[l, m, p, kt, f] = W1_e[kt*128+p, m*128+f]
        w1[l] = W1_e.reshape(KT, 128, MT1, 128).transpose(2, 1, 0, 3).astype(nbf)
        # w2[l, mo, p, kt, f] = W2[kt*128+p, mo*128+f]  (kt here is over DFF: 24)
        w2[l] = W2.reshape(MT1, 128, KT, 128).transpose(2, 1, 0, 3).astype(nbf)
        # biases: qb_hd [128,8] (d-major), kb_hd, ob [128,6], b1 [128,24], b2 [128,6]
        qb = np.zeros((128, H), np.float32)
        kb = np.zeros((128, H), np.float32)
        qb[:96] = bq.reshape(H, 96).T
        kb[:96] = bk.reshape(H, 96).T
        biasA[l, :, 0:8] = qb
        biasA[l, :, 8:16] = kb
        biasA[l, :, 16:22] = bo.reshape(KT, 128).T
        biasA[l, :, 22:46] = bm1.reshape(MT1, 128).T
        biasA[l, :, 46:52] = bm2.reshape(KT, 128).T
        if np.any(bv_e != 0):
            has_vbias = True
        vbias_rows[l, 0] = bv_e.astype(nbf)

    gf, bf = p['lnf_g'], p['lnf_b']
    Wh = p['Whead']
    Wh_e = gf[:, None] * Wh
    headb = bf @ Wh
    whead = Wh_e.reshape(KT, 128, FULL).transpose(1, 0, 2).astype(nbf)
    has_headb = bool(np.any(headb != 0))
    headb_row = headb[None, :].astype(nbf)

    # tables (padded to 128-row tiles), lat/lon emb 256 cols, sog/cog 128
    tabs = np.zeros((128, 7, 256), nbf)
    iota = np.zeros((128, 8), np.float32)
    tbls = [p['lat_emb'], p['lon_emb'], p['sog_emb'], p['cog_emb']]
    g = 0
    for a, tb in enumerate(tbls):
        nv, ne = tb.shape
        nkt = (nv + 127) // 128
        for k in range(nkt):
            rows = tb[k * 128:(k + 1) * 128]
            tabs[:rows.shape[0], g, :ne] = rows.astype(nbf)
            iota[:, g] = k * 128 + np.arange(128)
            g += 1
    assert g == 7

    pos = p['pos_emb'][0]  # [T, C]
    pos_t = pos.reshape(T, KT, 128).transpose(2, 1, 0).astype(np.float32)
    pos_t = np.ascontiguousarray(pos_t)  # [128, 6, 1024]

    kl = np.arange(128)[:, None]
    ql = np.arange(128)[None, :]
    masks = (ql >= kl).astype(nbf)  # [128,128] lower-causal in [k,q] layout

    out.update(wqkv=wqkv, wo=wo, w1=w1, w2=w2, whead=whead, biasA=biasA,
               tabs=tabs, iota=iota, pos=pos_t, masks=masks,
               vbias=vbias_rows, headb=headb_row,
               has_vbias=has_vbias, has_headb=has_headb)
    return out


# ---------------------------------------------------------------- builder
def build(n_layers=L, has_vbias=False, has_headb=False):
    nc = bass.Bass()
    d = {}
    d['wqkv'] = nc.declare_dram_parameter('wqkv', [L, 3, 128, KT, C], BF16)
    d['wo'] = nc.declare_dram_parameter('wo', [L, 96, H, C], BF16)
    d['w1'] = nc.declare_dram_parameter('w1', [L, MT1, 128, KT, 128], BF16)
    d['w2'] = nc.declare_dram_parameter('w2', [L, KT, 128, MT1, 128], BF16)
    d['whead'] = nc.declare_dram_parameter('whead', [128, KT, FULL], BF16)
    d['biasA'] = nc.declare_dram_parameter('biasA', [L, 128, 52], F32)
    d['tabs'] = nc.declare_dram_parameter('tabs', [128, 7, 256], BF16)
    d['iota'] = nc.declare_dram_parameter('iota', [128, 8], F32)
    d['pos'] = nc.declare_dram_parameter('pos', [128, KT, T], F32)
    d['masks'] = nc.declare_dram_parameter('masks', [128, 128], BF16)
    d['idx'] = nc.declare_dram_parameter('idx', [32, 512], F32)
    if has_vbias:
        d['vbias'] = nc.declare_dram_parameter('vbias', [L, 1, C], BF16)
    if has_headb:
        d['headb'] = nc.declare_dram_parameter('headb', [1, FULL], BF16)
    out_d = nc.declare_dram_parameter('out', [32, 128, FULL], F32, isOutput=True)
    hbuf = nc.dram_tensor('hbuf', [KT, 128, TOK], F32)

    with tile.TileContext(nc) as tc:
        from contextlib import ExitStack
        with ExitStack() as ctx:
            sb = ctx.enter_context(tc.tile_pool(name='sb', bufs=1))
            sb2 = ctx.enter_context(tc.tile_pool(name='sb2', bufs=2))
            sb3 = ctx.enter_context(tc.tile_pool(name='sb3', bufs=3))
            sb4 = ctx.enter_context(tc.tile_pool(name='sb4', bufs=3))
            stats = ctx.enter_context(tc.tile_pool(name='stats', bufs=4))
            ln7 = ctx.enter_context(tc.tile_pool(name='ln7', bufs=6))
            gel = ctx.enter_context(tc.tile_pool(name='gel', bufs=24))
            pp_ = ctx.enter_context(tc.tile_pool(name='pp', bufs=4))
            ps_small = ctx.enter_context(
                tc.tile_pool(name='ps_s', bufs=2, space='PSUM'))
            ps_b = ctx.enter_context(
                tc.tile_pool(name='ps_b', bufs=2, space='PSUM'))
            ps_mm = ctx.enter_context(
                tc.tile_pool(name='ps_mm', bufs=4, space='PSUM'))

            # constants
            onesC = sb.tile([128, 1], F32, tag='onesC')
            nc.vector.memset(onesC[:], 1.0 / C)
            ones_row = sb.tile([1, 128], F32, tag='ones_row')
            nc.vector.memset(ones_row[:], 1.0)
            ones_row_bf = sb.tile([1, 128], BF16, tag='ones_row_bf')
            nc.vector.memset(ones_row_bf[:], 1.0)
            eps_sb = sb.tile([1, 1], F32, tag='eps')
            nc.vector.memset(eps_sb[:], EPS)

            masks_sb = sb.tile([128, 128], BF16, tag='masks')
            nc.sync.dma_start(out=masks_sb[:], in_=d['masks'][:])
            tabs_sb = sb.tile([128, 7, 256], BF16, tag='tabs')
            nc.sync.dma_start(out=tabs_sb[:], in_=d['tabs'][:])
            iota_sb = sb.tile([128, 8], F32, tag='iota')
            nc.sync.dma_start(out=iota_sb[:], in_=d['iota'][:])

            # ---------------- embedding ----------------
            for c in range(NC_CHUNK):
                csl = slice(c * 512, (c + 1) * 512)
                ct_base = 0
                for a, groups in TAB_GROUPS:
                    r = a * 8 + c
                    idxb = sb2.tile([128, 512], F32, tag='idxb')
                    nc.sync.dma_start(
                        out=idxb[:], in_=d['idx'][r:r + 1, :].broadcast_to((128, 512)))
                    ohs = []
                    for g in groups:
                        oh = sb2.tile([128, 512], BF16, tag='onehot')
                        nc.vector.tensor_scalar(
                            out=oh[:], in0=idxb[:],
                            scalar1=iota_sb[:, g:g + 1], scalar2=None,
                            op0=OP.is_equal)
                        ohs.append(oh)
                    n_m = EMB_SIZES[a] // 128
                    for mi in range(n_m):
                        pe = ps_mm.tile([128, 512], F32, tag='ps_mm')
                        for j, (g, oh) in enumerate(zip(groups, ohs)):
                            nc.tensor.matmul(
                                pe[:], tabs_sb[:, g, mi * 128:(mi + 1) * 128],
                                oh[:], start=(j == 0), stop=(j == len(groups) - 1))
                        ct = ct_base + mi
                        post = sb4.tile([128, 512], F32, tag='hin')
                        nc.sync.dma_start(
                            out=post[:],
                            in_=d['pos'][:, ct, (c % 2) * 512:(c % 2) * 512 + 512])
                        h0 = sb4.tile([128, 512], F32, tag='hin')
                        nc.vector.scalar_tensor_tensor(
                            out=h0[:], in0=pe[:], scalar=1.0, in1=post[:],
                            op0=OP.mult, op1=OP.add)
                        nc.sync.dma_start(out=hbuf[ct_base + mi, :, csl], in_=h0[:])
                    ct_base += n_m

            whead_sb = sb.tile([128, KT, FULL], BF16, tag='tabs')
            for kt in range(KT):
                nc.sync.dma_start(out=whead_sb[:, kt, :], in_=d['whead'][:, kt, :])
            if has_headb:
                headb_sb = sb.tile([1, FULL], BF16, tag='headb')
                nc.sync.dma_start(out=headb_sb[:], in_=d['headb'][:])

            # ---------------- helper: layernorm into xn tile ----------------
            def layer_norm(c, xn):
                """Reads hbuf chunk c, writes standardized bf16 into xn
                ([128, KT, 512]). Returns nothing."""
                csl = slice(c * 512, (c + 1) * 512)
                hts = []
                mu_ps = ps_small.tile([1, 512], F32, tag='ps_small')
                ex2_ps = ps_small.tile([1, 512], F32, tag='ps_small')
                for kt in range(KT):
                    ht = ln7.tile([128, 512], F32, tag='hln')
                    nc.sync.dma_start(out=ht[:], in_=hbuf[kt, :, csl])
                    hts.append(ht)
                    nc.tensor.matmul(mu_ps[:], onesC[:], ht[:],
                                     start=(kt == 0), stop=(kt == KT - 1))
                for kt in range(KT):
                    x2 = sb2.tile([128, 512], F32, tag='x2')
                    nc.vector.scalar_tensor_tensor(
                        out=x2[:], in0=hts[kt][:], scalar=1.0, in1=hts[kt][:],
                        op0=OP.mult, op1=OP.mult)
                    nc.tensor.matmul(ex2_ps[:], onesC[:], x2[:],
                                     start=(kt == 0), stop=(kt == KT - 1))
                mu_sb = stats.tile([1, 512], F32, tag='st')
                nc.vector.tensor_copy(mu_sb[:], mu_ps[:])
                t1 = stats.tile([1, 512], F32, tag='st')
                nc.vector.tensor_tensor(out=t1[:], in0=mu_sb[:], in1=mu_sb[:],
                                        op=OP.mult)
                var = stats.tile([1, 512], F32, tag='st')
                nc.vector.tensor_tensor(out=var[:], in0=ex2_ps[:], in1=t1[:],
                                        op=OP.subtract)
                std = stats.tile([1, 512], F32, tag='st')
                nc.scalar.activation(std[:], var[:], AF.Sqrt, bias=eps_sb[:])
                inv = stats.tile([1, 512], F32, tag='st')
                nc.vector.reciprocal(inv[:], std[:])
                mub = ps_b.tile([128, 512], F32, tag='ps_bc')
                nc.tensor.matmul(mub[:], ones_row[:], mu_sb[:])
                sbc = ps_b.tile([128, 512], F32, tag='ps_bc')
                nc.tensor.matmul(sbc[:], ones_row[:], inv[:])
                for kt in range(KT):
                    cent = sb2.tile([128, 512], F32, tag='x2')
                    nc.vector.scalar_tensor_tensor(
                        out=cent[:], in0=hts[kt][:], scalar=1.0, in1=mub[:],
                        op0=OP.mult, op1=OP.subtract)
                    nc.vector.tensor_tensor(out=xn[:, kt, :], in0=cent[:],
                                            in1=sbc[:], op=OP.mult)

            # ---------------- layers ----------------
            for l in range(n_layers):
                wq_sb = sb.tile([128, KT, C], BF16, tag='wq')
                wk_sb = sb.tile([128, KT, C], BF16, tag='wk')
                wv_sb = sb.tile([128, KT, C], BF16, tag='wv')
                for kt in range(KT):
                    nc.sync.dma_start(out=wq_sb[:, kt, :], in_=d['wqkv'][l, 0, :, kt, :])
                    nc.sync.dma_start(out=wk_sb[:, kt, :], in_=d['wqkv'][l, 1, :, kt, :])
                    nc.sync.dma_start(out=wv_sb[:, kt, :], in_=d['wqkv'][l, 2, :, kt, :])
                bias_sb = sb2.tile([128, 52], F32, tag='biasA')
                nc.sync.dma_start(out=bias_sb[:], in_=d['biasA'][l])
                if has_vbias:
                    vb = sb2.tile([1, C], BF16, tag='vbias')
                    nc.sync.dma_start(out=vb[:], in_=d['vbias'][l])

                # ---- attention, per sequence ----
                for s in range(SEQ_PER_CORE):
                    q_hd = sb.tile([128, H, T], BF16, tag='q_hd')
                    k_hd = sb.tile([128, H, T], BF16, tag='k_hd')
                    v_sb = sb.tile([128, 8, 776], BF16, tag='v_sb')
                    y_hd = sb.tile([128, H, T], BF16, tag='y_hd')
                    nc.vector.memset(v_sb[:, :, 768:776], 1.0)
                    for cc in range(2):
                        c = s * 2 + cc
                        xn = sb3.tile([128, KT, 512], BF16, tag='xn')
                        layer_norm(c, xn)
                        tsl = slice(cc * 512, cc * 512 + 512)
                        for h in range(H):
                            qp = ps_mm.tile([128, 512], F32, tag='ps_mm')
                            for kt in range(KT):
                                nc.tensor.matmul(
                                    qp[0:96, :],
                                    wq_sb[:, kt, h * 96:(h + 1) * 96],
                                    xn[:, kt, :],
                                    start=(kt == 0), stop=(kt == KT - 1))
                            nc.vector.tensor_scalar(
                                out=q_hd[0:96, h, tsl], in0=qp[0:96, :],
                                scalar1=bias_sb[0:96, h:h + 1], scalar2=None,
                                op0=OP.add)
                            kp = ps_mm.tile([128, 512], F32, tag='ps_mm')
                            for kt in range(KT):
                                nc.tensor.matmul(
                                    kp[0:96, :],
                                    wk_sb[:, kt, h * 96:(h + 1) * 96],
                                    xn[:, kt, :],
                                    start=(kt == 0), stop=(kt == KT - 1))
                            nc.vector.tensor_scalar(
                                out=k_hd[0:96, h, tsl], in0=kp[0:96, :],
                                scalar1=bias_sb[0:96, 8 + h:9 + h], scalar2=None,
                                op0=OP.add)
                        for tm in range(4):
                            for (n0, nw) in ((0, 512), (512, 256)):
                                vp = ps_mm.tile([128, 512], F32, tag='ps_mm')
                                for kt in range(KT):
                                    nc.tensor.matmul(
                                        vp[:, 0:nw],
                                        xn[:, kt, tm * 128:(tm + 1) * 128],
                                        wv_sb[:, kt, n0:n0 + nw],
                                        start=(kt == 0),
                                        stop=(kt == KT - 1 if not has_vbias else False))
                                if has_vbias:
                                    nc.tensor.matmul(
                                        vp[:, 0:nw], ones_row_bf[:],
                                        vb[:, n0:n0 + nw],
                                        start=False, stop=True)
                                nc.vector.tensor_copy(
                                    v_sb[:, cc * 4 + tm, n0:n0 + nw], vp[:, 0:nw])
                    # attention proper
                    for h in range(H):
                        for qn in range(2):
                            nkt = 4 * (qn + 1)
                            qsl = slice(qn * 512, qn * 512 + 512)
                            yp = ps_mm.tile([128, 512], F32, tag='ps_mm')
                            for kt in range(nkt):
                                sp = ps_mm.tile([128, 512], F32, tag='ps_mm')
                                nc.tensor.matmul(
                                    sp[:], k_hd[0:96, h, kt * 128:(kt + 1) * 128],
                                    q_hd[0:96, h, qsl])
                                p_t = pp_.tile([128, 512], BF16, tag='p')
                                nc.scalar.activation(p_t[:], sp[:], AF.Exp)
                                j = kt - 4 * qn
                                if 0 <= j:
                                    nc.vector.tensor_tensor(
                                        out=p_t[:, 128 * j:128 * j + 128],
                                        in0=p_t[:, 128 * j:128 * j + 128],
                                        in1=masks_sb[:], op=OP.mult)
                                    if j > 0:
                                        nc.vector.memset(p_t[:, 0:128 * j], 0.0)
                                nc.tensor.matmul(
                                    yp[0:97, :], v_sb[:, kt, h:776:8], p_t[:],
                                    start=(kt == 0), stop=(kt == nkt - 1))
                            rec = stats.tile([1, 512], F32, tag='st')
                            nc.vector.reciprocal(rec[:], yp[96:97, :])
                            rb = ps_b.tile([128, 512], F32, tag='ps_bc')
                            nc.tensor.matmul(rb[0:96, :], ones_row[0:1, 0:96], rec[:])
                            rbs = sb2.tile([128, 512], F32, tag='rbs')
                            nc.vector.tensor_copy(rbs[0:96, :], rb[0:96, :])
                            nc.vector.tensor_tensor(
                                out=y_hd[0:96, h, qsl], in0=yp[0:96, :],
                                in1=rbs[0:96, :], op=OP.mult)
                    # projection + residual
                    for qn in range(2):
                        c = s * 2 + qn
                        csl = slice(c * 512, (c + 1) * 512)
                        qsl = slice(qn * 512, qn * 512 + 512)
                        for mo in range(KT):
                            wos = sb3.tile([96, H, 128], BF16, tag='wos')
                            nc.sync.dma_start(
                                out=wos[:], in_=d['wo'][l, :, :, mo * 128:(mo + 1) * 128])
                            pj = ps_mm.tile([128, 512], F32, tag='ps_mm')
                            for ht in range(H):
                                nc.tensor.matmul(
                                    pj[:], wos[:, ht, :],
                                    y_hd[0:96, ht, qsl],
                                    start=(ht == 0), stop=(ht == H - 1))
                            hres = sb4.tile([128, 512], F32, tag='hin')
                            nc.sync.dma_start(out=hres[:], in_=hbuf[mo, :, csl])
                            hnew = sb4.tile([128, 512], F32, tag='hin')
                            nc.vector.scalar_tensor_tensor(
                                out=hnew[:], in0=pj[:],
                                scalar=bias_sb[:, 16 + mo:17 + mo], in1=hres[:],
                                op0=OP.add, op1=OP.add)
                            nc.sync.dma_start(out=hbuf[mo, :, csl], in_=hnew[:])

                # ---- MLP, per 512-token chunk ----
                for c in range(NC_CHUNK):
                    csl = slice(c * 512, (c + 1) * 512)
                    xn = sb3.tile([128, KT, 512], BF16, tag='xn')
                    layer_norm(c, xn)
                    gts = []
                    for m in range(MT1):
                        w1s = sb4.tile([128, KT, 128], BF16, tag='w1s')
                        nc.sync.dma_start(out=w1s[:], in_=d['w1'][l, m])
                        mp = ps_mm.tile([128, 512], F32, tag='ps_mm')
                        for kt in range(KT):
                            nc.tensor.matmul(mp[:], w1s[:, kt, :], xn[:, kt, :],
                                             start=(kt == 0), stop=(kt == KT - 1))
                        g = gel.tile([128, 512], BF16, tag='g')
                        nc.scalar.activation(g[:], mp[:], AF.Gelu,
                                             bias=bias_sb[:, 22 + m:23 + m])
                        gts.append(g)
                    for mo in range(KT):
                        w2s = sb2.tile([128, MT1, 128], BF16, tag='w2s')
                        nc.sync.dma_start(out=w2s[:], in_=d['w2'][l, mo])
                        op_ = ps_mm.tile([128, 512], F32, tag='ps_mm')
                        for kt in range(MT1):
                            nc.tensor.matmul(op_[:], w2s[:, kt, :], gts[kt][:],
                                             start=(kt == 0), stop=(kt == MT1 - 1))
                        hres = sb4.tile([128, 512], F32, tag='hin')
                        nc.sync.dma_start(out=hres[:], in_=hbuf[mo, :, csl])
                        hnew = sb4.tile([128, 512], F32, tag='hin')
                        nc.vector.scalar_tensor_tensor(
                            out=hnew[:], in0=op_[:],
                            scalar=bias_sb[:, 46 + mo:47 + mo], in1=hres[:],
                            op0=OP.add, op1=OP.add)
                        nc.sync.dma_start(out=hbuf[mo, :, csl], in_=hnew[:])

            # ---------------- final LN + head ----------------
            for c in range(NC_CHUNK):
                xn = sb3.tile([128, KT, 512], BF16, tag='xn')
                layer_norm(c, xn)
                for tm in range(4):
                    ob_t = sb.tile([128, FULL], F32, tag='outsb')
                    for (n0, nw) in ((0, 512), (512, FULL - 512)):
                        hp = ps_mm.tile([128, 512], F32, tag='ps_mm')
                        for kt in range(KT):
                            nc.tensor.matmul(
                                hp[:, 0:nw],
                                xn[:, kt, tm * 128:(tm + 1) * 128],
                                whead_sb[:, kt, n0:n0 + nw],
                                start=(kt == 0),
                                stop=(kt == KT - 1 if not has_headb else False))
                        if has_headb:
                            nc.tensor.matmul(hp[:, 0:nw], ones_row_bf[:],
                                             headb_sb[:, n0:n0 + nw],
                                             start=False, stop=True)
                        nc.vector.tensor_copy(ob_t[:, n0:n0 + nw], hp[:, 0:nw])
                    nc.sync.dma_start(out=out_d[c * 4 + tm], in_=ob_t[:])

    return nc


_CACHE = {}
_LAST_RESULT = None


def kernel(x, params):
    x = np.asarray(x, np.float32)
    prep = prep_params(params)
    key = (L, prep['has_vbias'], prep['has_headb'])
    if key not in _CACHE:
        _CACHE[key] = build(L, prep['has_vbias'], prep['has_headb'])
    nc = _CACHE[key]

    idx_full = np.floor(x * np.asarray(ATT_SIZES, np.float32)).astype(np.float32)
    # reference casts via .astype(int32) which truncates; x>=0 so floor==trunc
    in_maps = []
    for core in range(N_CORES):
        xs = idx_full[core * SEQ_PER_CORE:(core + 1) * SEQ_PER_CORE]  # [4,1024,4]
        flat = xs.reshape(TOK, 4).T  # [4, 4096] attr-major
        idx_rows = flat.reshape(4, NC_CHUNK, 512).reshape(32, 512)
        m = dict(wqkv=prep['wqkv'], wo=prep['wo'], w1=prep['w1'], w2=prep['w2'],
                 whead=prep['whead'], biasA=prep['biasA'], tabs=prep['tabs'],
                 iota=prep['iota'], pos=prep['pos'], masks=prep['masks'],
                 idx=np.ascontiguousarray(idx_rows))
        if prep['has_vbias']:
            m['vbias'] = prep['vbias']
        if prep['has_headb']:
            m['headb'] = prep['headb']
        in_maps.append(m)

    import os
    trace = bool(os.environ.get("BASS_TRACE"))
    res = run_bass_kernel_spmd(nc, in_maps, list(range(N_CORES)), trace=trace)
    global _LAST_RESULT
    _LAST_RESULT = res
    outs = []
    for core in range(N_CORES):
        o = res.results[core]['out']  # [32, 128, 622]
        outs.append(o.reshape(SEQ_PER_CORE, T, FULL))
    return np.concatenate(outs, axis=0)
